# revision 33
# baseline (speedup 1.0000x reference)
"""Trainium2 Bass kernel for nn_AdaptiveGraphLearning (topk_masking).

Math (after simplification of the reference):
  Only chunk i=0 of the reference loop runs: qc = full q (B,H,N,32),
  kc = k of the FIRST 1024 nodes. Soft-threshold is identity.
    scores(n,u) = sum_o relu(co_o(n,u)) + t(n,u),   u in [0,1024)
  The per-head logits fold into host-precomputed (65x65) matrices via
  the ones-augmentation trick (biases handled exactly):
    co_o = x~ A_o x~^T,  t = x~ A_t x~^T,  x~ = [x | 1]
  With relu(x) = x/2 + |x|/2:
    scores = T + |C_1| + |C_2| + |C_3| + |C_4|
  where C_o = x~ (A_o/2) x~^T and T = x~ (A_t + sum_o A_o / 2) x~^T.
  Output adj[b,n,:] = scores masked to the row's top-32 entries
  (scatter-add of distinct top_k indices == masked copy); columns
  1024..2047 stay zero.

Split across host/device:
  host:   fold weights into the 5 (65x65) matrices; y_v = x~ @ A_v
          (tiny stage-1 GEMM, fp64); split into half-precision hi/lo
          pairs for full-rate PE matmuls with fp32 PSUM accumulate:
          T-variant bf16x3 (hi*hi + hi*lo + lo*hi, ~1e-5 accurate),
          C-variants fp16x2 (hi*hi + hi*lo, ~2^-11 accurate - enough
          because |C_o| only shifts top-32 picks on near-ties).
  device (per core, batch-parallel over 8 cores, no collectives):
    per 128-row tile x 512-col chunk:
      PE:  3 bf16 matmuls (T) + 2 fp16 matmuls x4 (C) -> 5 PSUM banks
      ACT: |C_o| evac PSUM->SBUF (4x)
      DVE: a12 = |C1|+|C2|, a34 = |C3|+|C4|
      then either (3 of 4 chunks) PE identity-matmul-accumulates a12,
      a34 onto T's bank + ACT copies scores out, or (1 of 4) DVE adds
      them straight onto T.
    per tile:
      DVE: top-32 threshold: top-8 per 64-wide segment (16x max8) then
           top-32 of the 128 candidates (4x max8 + 3x match_replace);
           exact unless a segment holds >8 of the row's top-32
           (17/16384 rows on the real inputs).
      ACT: r = relu(scores - (t32 - eps))   [masked, values shifted]
      DMA: r -> out left half; t32 column -> tvals
  host:   out = where(r > 0, r + (t32 - eps), 0); right half zeros
          (runner pre-zeros output buffers; host also re-zeros).
"""

import sys

import numpy as np

try:
    import concourse  # noqa: F401
except ImportError:  # grading env: concourse lives in /opt/trn_rl_repo
    sys.path.insert(0, "/opt/trn_rl_repo")

B, N, IN_DIM = 8, 2048, 64
HEADS, OUT_DIM = 4, 32
U = 1024  # only the first ceil(N/2) nodes appear as columns
KSEL = 32  # top-k per row
KDIM = IN_DIM + 1  # augmented contraction dim (65)
NV = 5  # T, C1..C4
N_CORES = 8
NTILES = N // 128  # 16
UCHUNK = 512
NU = U // UCHUNK  # 2
EPS = 1e-5
NSEG = 16  # top-k candidate segments per row
SEGW = U // NSEG  # 64
DVE_ASSEMBLY_FRAC = (1, 4)  # (num, den): num of every den chunks assemble on DVE

_compiled = None


def _build_a_matrices(Wq, bq, Wk, bk, mlp_w, mlp_b):
    """Return A (5,65,65) float64: A[0]=T-matrix, A[1..4]=C_o matrices."""
    inv = 1.0 / np.sqrt(OUT_DIM)
    Ao = np.zeros((HEADS, KDIM, KDIM))
    At = np.zeros((KDIM, KDIM))
    for h in range(HEADS):
        sl = slice(h * OUT_DIM, (h + 1) * OUT_DIM)
        Wq_h = Wq[sl, :].astype(np.float64)
        Wk_h = Wk[sl, :].astype(np.float64)
        bq_h = bq[sl].astype(np.float64)
        bk_h = bk[sl].astype(np.float64)
        Ah = np.zeros((KDIM, KDIM))
        Ah[:IN_DIM, :IN_DIM] = Wq_h.T @ Wk_h
        Ah[IN_DIM, :IN_DIM] = bq_h @ Wk_h
        Ah[:IN_DIM, IN_DIM] = Wq_h.T @ bk_h
        Ah[IN_DIM, IN_DIM] = bq_h @ bk_h
        for o in range(HEADS):
            Ao[o] += mlp_w[o, h] * inv * Ah
        At += inv * Ah
    for o in range(HEADS):
        Ao[o][IN_DIM, IN_DIM] += mlp_b[o]
    A = np.zeros((NV, KDIM, KDIM))
    A[0] = At + 0.5 * Ao.sum(axis=0)  # T-variant
    for o in range(HEADS):
        A[o + 1] = 0.5 * Ao[o]  # C-variants (|.| applied on device)
    return A


def _kernel_body(nc, tc, ins, outs, ctx):
    import concourse.mybir as mybir

    f32 = mybir.dt.float32
    bf16 = mybir.dt.bfloat16
    f16 = mybir.dt.float16
    Abs = mybir.ActivationFunctionType.Abs
    Relu = mybir.ActivationFunctionType.Relu
    Alu = mybir.AluOpType
    yth_d, ytl_d, ych_d, xbh_d, xbl_d, xfh_d, xfl_d, id_d = ins
    out_d, tv_d = outs

    const = ctx.enter_context(tc.tile_pool(name="const", bufs=1))
    psum = ctx.enter_context(tc.tile_pool(name="psum", bufs=8, space="PSUM"))
    apool = ctx.enter_context(tc.tile_pool(name="a", bufs=8))
    spool = ctx.enter_context(tc.tile_pool(name="scores", bufs=3))
    wpool = ctx.enter_context(tc.tile_pool(name="w", bufs=2))
    mxpool = ctx.enter_context(tc.tile_pool(name="mx", bufs=3))
    opool = ctx.enter_context(tc.tile_pool(name="o", bufs=3))

    xbh = const.tile([KDIM, U], bf16, tag="xbh")
    xbl = const.tile([KDIM, U], bf16, tag="xbl")
    xfh = const.tile([KDIM, U], f16, tag="xfh")
    xfl = const.tile([KDIM, U], f16, tag="xfl")
    ident = const.tile([128, 128], f32, tag="id")
    nc.sync.dma_start(xbh[:], xbh_d[:])
    nc.sync.dma_start(xbl[:], xbl_d[:])
    nc.sync.dma_start(xfh[:], xfh_d[:])
    nc.sync.dma_start(xfl[:], xfl_d[:])
    nc.sync.dma_start(ident[:], id_d[:])
    yth = const.tile([KDIM, N], bf16, tag="yth")
    ytl = const.tile([KDIM, N], bf16, tag="ytl")
    nc.sync.dma_start(yth[:], yth_d[:])
    nc.sync.dma_start(ytl[:], ytl_d[:])
    ycv = []
    for v in range(4):
        tv = const.tile([KDIM, N], f16, tag=f"yc{v}", name=f"yc{v}")
        nc.sync.dma_start(tv[:], ych_d[:, v * N:(v + 1) * N])
        ycv.append(tv)

    for n in range(NTILES):
        rs = n * 128
        scores = spool.tile([128, U], f32, tag="s")
        mx = mxpool.tile([128, 32], f32, tag="mx")
        for u in range(NU):
            us = u * UCHUNK
            chunk_id = n * NU + u
            dve_asm = chunk_id % DVE_ASSEMBLY_FRAC[1] < DVE_ASSEMBLY_FRAC[0]
            ysl = slice(rs, rs + 128)
            usl = slice(us, us + UCHUNK)
            # T-variant: bf16x3 into bank 0
            bt = psum.tile([128, UCHUNK], f32, tag="ps", name="bt")
            nc.tensor.matmul(bt[:], yth[:, ysl], xbh[:, usl],
                             start=True, stop=False)
            nc.tensor.matmul(bt[:], yth[:, ysl], xbl[:, usl],
                             start=False, stop=False)
            nc.tensor.matmul(bt[:], ytl[:, ysl], xbh[:, usl],
                             start=False, stop=dve_asm)
            # C-variants: fp16x2 into banks 1..4
            banks = [bt]
            for v in range(4):
                bk = psum.tile([128, UCHUNK], f32, tag="ps", name="bank")
                nc.tensor.matmul(bk[:], ycv[v][:, ysl], xfh[:, usl],
                                 start=True, stop=False)
                nc.tensor.matmul(bk[:], ycv[v][:, ysl], xfl[:, usl],
                                 start=False, stop=True)
                banks.append(bk)
            # |C_o| evacuations on ACT
            av = []
            for v in range(1, NV):
                a = apool.tile([128, UCHUNK], f32, tag="a", name="aabs")
                nc.scalar.activation(a[:], banks[v][:], Abs)
                av.append(a)
            # pairwise sums on DVE
            a12 = apool.tile([128, UCHUNK], f32, tag="a12")
            a34 = apool.tile([128, UCHUNK], f32, tag="a34")
            nc.vector.tensor_tensor(out=a12[:], in0=av[0][:], in1=av[1][:], op=Alu.add)
            nc.vector.tensor_tensor(out=a34[:], in0=av[2][:], in1=av[3][:], op=Alu.add)
            if dve_asm:
                # assemble on DVE: scores = (a12 + a34) + T(psum)
                s12 = apool.tile([128, UCHUNK], f32, tag="s12")
                nc.vector.tensor_tensor(out=s12[:], in0=a12[:], in1=a34[:], op=Alu.add)
                nc.vector.tensor_tensor(out=scores[:, usl],
                                        in0=s12[:], in1=bt[:], op=Alu.add)
            else:
                # accumulate onto T's bank via identity matmuls (fp32)
                nc.tensor.matmul(bt[:], ident[:], a12[:], start=False, stop=False)
                nc.tensor.matmul(bt[:], ident[:], a34[:], start=False, stop=True)
                nc.scalar.copy(scores[:, usl], bt[:])

        # top-32 via segmented candidates: top-8 per 64-wide segment, then
        # top-32 of the 128 candidates (exact unless a segment holds >8 of
        # the row's top-32; 17/16384 rows on the real inputs).
        cand = wpool.tile([128, 128], f32, tag="cand")
        for s in range(NSEG):
            nc.vector.max(out=cand[:, s * 8:(s + 1) * 8],
                          in_=scores[:, s * SEGW:(s + 1) * SEGW])
        w = wpool.tile([128, 128], f32, tag="w")
        nc.vector.max(out=mx[:, 0:8], in_=cand[:])
        nc.vector.match_replace(out=w[:], in_to_replace=mx[:, 0:8],
                                in_values=cand[:], imm_value=-1e30)
        for r in range(1, 3):
            nc.vector.max(out=mx[:, r * 8:(r + 1) * 8], in_=w[:])
            nc.vector.match_replace(out=w[:], in_to_replace=mx[:, r * 8:(r + 1) * 8],
                                    in_values=w[:], imm_value=-1e30)
        nc.vector.max(out=mx[:, 24:32], in_=w[:])

        # nt = -(t32 - eps); r = relu(scores + nt) on ACT
        nt = mxpool.tile([128, 1], f32, tag="nt")
        nc.vector.tensor_scalar(out=nt[:], in0=mx[:, 31:32], scalar1=-1.0,
                                scalar2=EPS, op0=Alu.mult, op1=Alu.add)
        msk = opool.tile([128, U], f32, tag="o")
        nc.scalar.activation(msk[:], scores[:], Relu, bias=nt[:, 0:1], scale=1.0)
        # split the 512KB output store across queues for DMA-engine parallelism
        for p, eng in ((0, nc.sync), (1, nc.gpsimd), (2, nc.sync), (3, nc.gpsimd)):
            eng.dma_start(out_d[rs + p * 32:rs + (p + 1) * 32, 0:U],
                          msk[p * 32:(p + 1) * 32, :])
        nc.sync.dma_start(tv_d[rs:rs + 128, 0:1], mx[:, 31:32])


def _build_nc():
    from contextlib import ExitStack

    import concourse.mybir as mybir
    import concourse.tile as tile
    from concourse import bacc

    f32 = mybir.dt.float32
    bf16 = mybir.dt.bfloat16
    f16 = mybir.dt.float16
    nc = bacc.Bacc(
        "TRN2", target_bir_lowering=False, debug=False, num_devices=N_CORES
    )
    yth_d = nc.dram_tensor("yth", [KDIM, N], bf16, kind="ExternalInput").ap()
    ytl_d = nc.dram_tensor("ytl", [KDIM, N], bf16, kind="ExternalInput").ap()
    ych_d = nc.dram_tensor("ych", [KDIM, 4 * N], f16, kind="ExternalInput").ap()
    xbh_d = nc.dram_tensor("xbh", [KDIM, U], bf16, kind="ExternalInput").ap()
    xbl_d = nc.dram_tensor("xbl", [KDIM, U], bf16, kind="ExternalInput").ap()
    xfh_d = nc.dram_tensor("xfh", [KDIM, U], f16, kind="ExternalInput").ap()
    xfl_d = nc.dram_tensor("xfl", [KDIM, U], f16, kind="ExternalInput").ap()
    id_d = nc.dram_tensor("ident", [128, 128], f32, kind="ExternalInput").ap()
    out_d = nc.dram_tensor("out", [N, N], f32, kind="ExternalOutput").ap()
    tv_d = nc.dram_tensor("tvals", [N, 1], f32, kind="ExternalOutput").ap()
    with tile.TileContext(nc) as tc, ExitStack() as ctx:
        _kernel_body(
            nc, tc,
            [yth_d, ytl_d, ych_d, xbh_d, xbl_d, xfh_d, xfl_d, id_d],
            [out_d, tv_d], ctx,
        )
    nc.compile()
    return nc


def _get_compiled():
    global _compiled
    if _compiled is None:
        _compiled = _build_nc()
    return _compiled


def _split(a, dt):
    hi = a.astype(dt)
    lo = (a - hi.astype(np.float64)).astype(dt)
    return np.ascontiguousarray(hi), np.ascontiguousarray(lo)


def kernel(x, Wq, bq, Wk, bk, mlp_w, mlp_b, ln_g, ln_b, _want_profile=False):
    import ml_dtypes

    from concourse.bass_utils import run_bass_kernel_spmd

    x = np.asarray(x, np.float32)
    A = _build_a_matrices(
        np.asarray(Wq), np.asarray(bq), np.asarray(Wk), np.asarray(bk),
        np.asarray(mlp_w), np.asarray(mlp_b),
    )  # (5,65,65) float64
    ident = np.eye(128, dtype=np.float32)

    xa = np.concatenate(
        [x.astype(np.float64), np.ones((B, N, 1))], axis=-1)  # (B,N,65)
    # host stage-1: yT[v] = (x~ @ A_v)^T per batch
    yt = np.einsum("vkm,bnk->bvmn", A, xa)  # (B,5,65,2048)
    in_maps = []
    for b in range(B):
        yth_, ytl_ = _split(yt[b, 0], ml_dtypes.bfloat16)
        ych_ = np.ascontiguousarray(
            yt[b, 1:].transpose(1, 0, 2).reshape(KDIM, 4 * N).astype(np.float16))
        xtb = xa[b, :U, :].T  # (65, 1024)
        xbh_, xbl_ = _split(xtb, ml_dtypes.bfloat16)
        xfh_, xfl_ = _split(xtb, np.float16)
        in_maps.append({
            "yth": yth_, "ytl": ytl_, "ych": ych_,
            "xbh": xbh_, "xbl": xbl_, "xfh": xfh_, "xfl": xfl_,
            "ident": ident,
        })

    nc = _get_compiled()
    res = run_bass_kernel_spmd(
        nc, in_maps, core_ids=list(range(N_CORES)), trace=_want_profile
    )
    out = np.zeros((B, N, N), np.float32)
    for b in range(B):
        r = res.results[b]["out"][:, :U]
        t32 = res.results[b]["tvals"][:, 0:1] - np.float32(EPS)
        out[b, :, :U] = np.where(r > 0, r + t32, 0.0)
    if _want_profile:
        return out, res
    return out


# revision 34
# speedup vs baseline: 1.1713x; 1.1713x over previous
"""Trainium2 Bass kernel for nn_AdaptiveGraphLearning (topk_masking).

Math (after simplification of the reference):
  Only chunk i=0 of the reference loop runs: qc = full q (B,H,N,32),
  kc = k of the FIRST 1024 nodes. Soft-threshold is identity.
    scores(n,u) = sum_o relu(co_o(n,u)) + t(n,u),   u in [0,1024)
  The per-head logits fold into host-precomputed (65x65) matrices via
  the ones-augmentation trick (biases handled exactly):
    co_o = x~ A_o x~^T,  t = x~ A_t x~^T,  x~ = [x | 1]
  With relu(x) = x/2 + |x|/2:
    scores = T + |C_1| + |C_2| + |C_3| + |C_4|
  where C_o = x~ (A_o/2) x~^T and T = x~ (A_t + sum_o A_o / 2) x~^T.
  Output adj[b,n,:] = scores masked to the row's top-32 entries
  (scatter-add of distinct top_k indices == masked copy); columns
  1024..2047 stay zero.

Split across host/device:
  host:   fold weights into the 5 (65x65) matrices; y_v = x~ @ A_v
          (tiny stage-1 GEMM, fp64); split into half-precision hi/lo
          pairs for full-rate PE matmuls with fp32 PSUM accumulate:
          T-variant bf16x3 (hi*hi + hi*lo + lo*hi, ~1e-5 accurate),
          C-variants fp16x2 (hi*hi + hi*lo, ~2^-11 accurate - enough
          because |C_o| only shifts top-32 picks on near-ties).
  device (per core, batch-parallel over 8 cores, no collectives):
    per 128-row tile x 512-col chunk:
      PE:  3 bf16 matmuls (T) + 2 fp16 matmuls x4 (C) -> 5 PSUM banks
      ACT: |C_o| evac PSUM->SBUF (4x)
      DVE: a12 = |C1|+|C2|, a34 = |C3|+|C4|
      then either (3 of 4 chunks) PE identity-matmul-accumulates a12,
      a34 onto T's bank + ACT copies scores out, or (1 of 4) DVE adds
      them straight onto T.
    per tile:
      DVE: top-32 threshold: top-8 per 64-wide segment (16x max8) then
           top-32 of the 128 candidates (4x max8 + 3x match_replace);
           exact unless a segment holds >8 of the row's top-32
           (17/16384 rows on the real inputs).
      ACT: r = relu(scores - (t32 - eps))   [masked, values shifted]
      DMA: r -> out left half; t32 column -> tvals
  host:   out = where(r > 0, r + (t32 - eps), 0); right half zeros
          (runner pre-zeros output buffers; host also re-zeros).
"""

import sys

import numpy as np

try:
    import concourse  # noqa: F401
except ImportError:  # grading env: concourse lives in /opt/trn_rl_repo
    sys.path.insert(0, "/opt/trn_rl_repo")

B, N, IN_DIM = 8, 2048, 64
HEADS, OUT_DIM = 4, 32
U = 1024  # only the first ceil(N/2) nodes appear as columns
KSEL = 32  # top-k per row
KDIM = IN_DIM + 1  # augmented contraction dim (65)
NV = 5  # T, C1..C4
N_CORES = 8
NTILES = N // 128  # 16
UCHUNK = 512
NU = U // UCHUNK  # 2
EPS = 1e-5
NSEG = 16  # top-k candidate segments per row
SEGW = U // NSEG  # 64
DVE_ASSEMBLY_FRAC = (1, 2)  # (num, den): num of every den chunks assemble on DVE

_compiled = None


def _build_a_matrices(Wq, bq, Wk, bk, mlp_w, mlp_b):
    """Return A (5,65,65) float64: A[0]=T-matrix, A[1..4]=C_o matrices."""
    inv = 1.0 / np.sqrt(OUT_DIM)
    Ao = np.zeros((HEADS, KDIM, KDIM))
    At = np.zeros((KDIM, KDIM))
    for h in range(HEADS):
        sl = slice(h * OUT_DIM, (h + 1) * OUT_DIM)
        Wq_h = Wq[sl, :].astype(np.float64)
        Wk_h = Wk[sl, :].astype(np.float64)
        bq_h = bq[sl].astype(np.float64)
        bk_h = bk[sl].astype(np.float64)
        Ah = np.zeros((KDIM, KDIM))
        Ah[:IN_DIM, :IN_DIM] = Wq_h.T @ Wk_h
        Ah[IN_DIM, :IN_DIM] = bq_h @ Wk_h
        Ah[:IN_DIM, IN_DIM] = Wq_h.T @ bk_h
        Ah[IN_DIM, IN_DIM] = bq_h @ bk_h
        for o in range(HEADS):
            Ao[o] += mlp_w[o, h] * inv * Ah
        At += inv * Ah
    for o in range(HEADS):
        Ao[o][IN_DIM, IN_DIM] += mlp_b[o]
    A = np.zeros((NV, KDIM, KDIM))
    A[0] = At + 0.5 * Ao.sum(axis=0)  # T-variant
    for o in range(HEADS):
        A[o + 1] = 0.5 * Ao[o]  # C-variants (|.| applied on device)
    return A


def _kernel_body(nc, tc, ins, outs, ctx):
    import concourse.mybir as mybir

    f32 = mybir.dt.float32
    bf16 = mybir.dt.bfloat16
    f16 = mybir.dt.float16
    Abs = mybir.ActivationFunctionType.Abs
    Relu = mybir.ActivationFunctionType.Relu
    Alu = mybir.AluOpType
    yth_d, ytl_d, ych_d, xbh_d, xbl_d, xfh_d, xfl_d, id_d = ins
    out_d, tv_d = outs

    const = ctx.enter_context(tc.tile_pool(name="const", bufs=1))
    psum = ctx.enter_context(tc.tile_pool(name="psum", bufs=8, space="PSUM"))
    apool = ctx.enter_context(tc.tile_pool(name="a", bufs=8))
    spool = ctx.enter_context(tc.tile_pool(name="scores", bufs=3))
    wpool = ctx.enter_context(tc.tile_pool(name="w", bufs=2))
    mxpool = ctx.enter_context(tc.tile_pool(name="mx", bufs=3))
    opool = ctx.enter_context(tc.tile_pool(name="o", bufs=3))

    xbh = const.tile([KDIM, U], bf16, tag="xbh")
    xbl = const.tile([KDIM, U], bf16, tag="xbl")
    xfh = const.tile([KDIM, U], f16, tag="xfh")
    xfl = const.tile([KDIM, U], f16, tag="xfl")
    ident = const.tile([128, 128], f32, tag="id")
    nc.sync.dma_start(xbh[:], xbh_d[:])
    nc.sync.dma_start(xbl[:], xbl_d[:])
    nc.sync.dma_start(xfh[:], xfh_d[:])
    nc.sync.dma_start(xfl[:], xfl_d[:])
    nc.sync.dma_start(ident[:], id_d[:])
    yth = const.tile([KDIM, N], bf16, tag="yth")
    ytl = const.tile([KDIM, N], bf16, tag="ytl")
    nc.sync.dma_start(yth[:], yth_d[:])
    nc.sync.dma_start(ytl[:], ytl_d[:])
    ycv = []
    for v in range(4):
        tv = const.tile([KDIM, N], f16, tag=f"yc{v}", name=f"yc{v}")
        nc.sync.dma_start(tv[:], ych_d[:, v * N:(v + 1) * N])
        ycv.append(tv)

    for n in range(NTILES):
        rs = n * 128
        scores = spool.tile([128, U], f32, tag="s")
        mx = mxpool.tile([128, 32], f32, tag="mx")
        for u in range(NU):
            us = u * UCHUNK
            chunk_id = n * NU + u
            dve_asm = chunk_id % DVE_ASSEMBLY_FRAC[1] < DVE_ASSEMBLY_FRAC[0]
            ysl = slice(rs, rs + 128)
            usl = slice(us, us + UCHUNK)
            # T-variant: bf16x3 into bank 0
            bt = psum.tile([128, UCHUNK], f32, tag="ps", name="bt")
            nc.tensor.matmul(bt[:], yth[:, ysl], xbh[:, usl],
                             start=True, stop=False)
            nc.tensor.matmul(bt[:], yth[:, ysl], xbl[:, usl],
                             start=False, stop=False)
            nc.tensor.matmul(bt[:], ytl[:, ysl], xbh[:, usl],
                             start=False, stop=dve_asm)
            # C-variants: fp16x2 into banks 1..4
            banks = [bt]
            for v in range(4):
                bk = psum.tile([128, UCHUNK], f32, tag="ps", name="bank")
                nc.tensor.matmul(bk[:], ycv[v][:, ysl], xfh[:, usl],
                                 start=True, stop=False)
                nc.tensor.matmul(bk[:], ycv[v][:, ysl], xfl[:, usl],
                                 start=False, stop=True)
                banks.append(bk)
            # |C_o| evacuations on ACT
            av = []
            for v in range(1, NV):
                a = apool.tile([128, UCHUNK], f32, tag="a", name="aabs")
                nc.scalar.activation(a[:], banks[v][:], Abs)
                av.append(a)
            # pairwise sums on DVE
            a12 = apool.tile([128, UCHUNK], f32, tag="a12")
            a34 = apool.tile([128, UCHUNK], f32, tag="a34")
            nc.vector.tensor_tensor(out=a12[:], in0=av[0][:], in1=av[1][:], op=Alu.add)
            nc.vector.tensor_tensor(out=a34[:], in0=av[2][:], in1=av[3][:], op=Alu.add)
            if dve_asm:
                # assemble on DVE: scores = (a12 + a34) + T(psum)
                s12 = apool.tile([128, UCHUNK], f32, tag="s12")
                nc.vector.tensor_tensor(out=s12[:], in0=a12[:], in1=a34[:], op=Alu.add)
                nc.vector.tensor_tensor(out=scores[:, usl],
                                        in0=s12[:], in1=bt[:], op=Alu.add)
            else:
                # accumulate onto T's bank via identity matmuls (fp32)
                nc.tensor.matmul(bt[:], ident[:], a12[:], start=False, stop=False)
                nc.tensor.matmul(bt[:], ident[:], a34[:], start=False, stop=True)
                nc.scalar.copy(scores[:, usl], bt[:])

        # top-32 via segmented candidates: top-8 per 64-wide segment, then
        # top-32 of the 128 candidates (exact unless a segment holds >8 of
        # the row's top-32; 17/16384 rows on the real inputs).
        cand = wpool.tile([128, 128], f32, tag="cand")
        for s in range(NSEG):
            nc.vector.max(out=cand[:, s * 8:(s + 1) * 8],
                          in_=scores[:, s * SEGW:(s + 1) * SEGW])
        w = wpool.tile([128, 128], f32, tag="w")
        nc.vector.max(out=mx[:, 0:8], in_=cand[:])
        nc.vector.match_replace(out=w[:], in_to_replace=mx[:, 0:8],
                                in_values=cand[:], imm_value=-1e30)
        for r in range(1, 3):
            nc.vector.max(out=mx[:, r * 8:(r + 1) * 8], in_=w[:])
            nc.vector.match_replace(out=w[:], in_to_replace=mx[:, r * 8:(r + 1) * 8],
                                    in_values=w[:], imm_value=-1e30)
        nc.vector.max(out=mx[:, 24:32], in_=w[:])

        # nt = -(t32 - eps); r = relu(scores + nt) on ACT
        nt = mxpool.tile([128, 1], f32, tag="nt")
        nc.vector.tensor_scalar(out=nt[:], in0=mx[:, 31:32], scalar1=-1.0,
                                scalar2=EPS, op0=Alu.mult, op1=Alu.add)
        msk = opool.tile([128, U], f32, tag="o")
        nc.scalar.activation(msk[:], scores[:], Relu, bias=nt[:, 0:1], scale=1.0)
        nc.sync.dma_start(out_d[rs:rs + 128, 0:U], msk[:])
        nc.sync.dma_start(tv_d[rs:rs + 128, 0:1], mx[:, 31:32])


def _build_nc():
    from contextlib import ExitStack

    import concourse.mybir as mybir
    import concourse.tile as tile
    from concourse import bacc

    f32 = mybir.dt.float32
    bf16 = mybir.dt.bfloat16
    f16 = mybir.dt.float16
    nc = bacc.Bacc(
        "TRN2", target_bir_lowering=False, debug=False, num_devices=N_CORES
    )
    yth_d = nc.dram_tensor("yth", [KDIM, N], bf16, kind="ExternalInput").ap()
    ytl_d = nc.dram_tensor("ytl", [KDIM, N], bf16, kind="ExternalInput").ap()
    ych_d = nc.dram_tensor("ych", [KDIM, 4 * N], f16, kind="ExternalInput").ap()
    xbh_d = nc.dram_tensor("xbh", [KDIM, U], bf16, kind="ExternalInput").ap()
    xbl_d = nc.dram_tensor("xbl", [KDIM, U], bf16, kind="ExternalInput").ap()
    xfh_d = nc.dram_tensor("xfh", [KDIM, U], f16, kind="ExternalInput").ap()
    xfl_d = nc.dram_tensor("xfl", [KDIM, U], f16, kind="ExternalInput").ap()
    id_d = nc.dram_tensor("ident", [128, 128], f32, kind="ExternalInput").ap()
    out_d = nc.dram_tensor("out", [N, N], f32, kind="ExternalOutput").ap()
    tv_d = nc.dram_tensor("tvals", [N, 1], f32, kind="ExternalOutput").ap()
    with tile.TileContext(nc) as tc, ExitStack() as ctx:
        _kernel_body(
            nc, tc,
            [yth_d, ytl_d, ych_d, xbh_d, xbl_d, xfh_d, xfl_d, id_d],
            [out_d, tv_d], ctx,
        )
    nc.compile()
    return nc


def _get_compiled():
    global _compiled
    if _compiled is None:
        _compiled = _build_nc()
    return _compiled


def _split(a, dt):
    hi = a.astype(dt)
    lo = (a - hi.astype(np.float64)).astype(dt)
    return np.ascontiguousarray(hi), np.ascontiguousarray(lo)


def kernel(x, Wq, bq, Wk, bk, mlp_w, mlp_b, ln_g, ln_b, _want_profile=False):
    import ml_dtypes

    from concourse.bass_utils import run_bass_kernel_spmd

    x = np.asarray(x, np.float32)
    A = _build_a_matrices(
        np.asarray(Wq), np.asarray(bq), np.asarray(Wk), np.asarray(bk),
        np.asarray(mlp_w), np.asarray(mlp_b),
    )  # (5,65,65) float64
    ident = np.eye(128, dtype=np.float32)

    xa = np.concatenate(
        [x.astype(np.float64), np.ones((B, N, 1))], axis=-1)  # (B,N,65)
    # host stage-1: yT[v] = (x~ @ A_v)^T per batch
    yt = np.einsum("vkm,bnk->bvmn", A, xa)  # (B,5,65,2048)
    in_maps = []
    for b in range(B):
        yth_, ytl_ = _split(yt[b, 0], ml_dtypes.bfloat16)
        ych_ = np.ascontiguousarray(
            yt[b, 1:].transpose(1, 0, 2).reshape(KDIM, 4 * N).astype(np.float16))
        xtb = xa[b, :U, :].T  # (65, 1024)
        xbh_, xbl_ = _split(xtb, ml_dtypes.bfloat16)
        xfh_, xfl_ = _split(xtb, np.float16)
        in_maps.append({
            "yth": yth_, "ytl": ytl_, "ych": ych_,
            "xbh": xbh_, "xbl": xbl_, "xfh": xfh_, "xfl": xfl_,
            "ident": ident,
        })

    nc = _get_compiled()
    res = run_bass_kernel_spmd(
        nc, in_maps, core_ids=list(range(N_CORES)), trace=_want_profile
    )
    out = np.zeros((B, N, N), np.float32)
    for b in range(B):
        r = res.results[b]["out"][:, :U]
        t32 = res.results[b]["tvals"][:, 0:1] - np.float32(EPS)
        out[b, :, :U] = np.where(r > 0, r + t32, 0.0)
    if _want_profile:
        return out, res
    return out
